# revision 43
# baseline (speedup 1.0000x reference)
"""AgentGNN (2x CGConv + BN + residual + ReLU) on 8 TRN2 NeuronCores.

Self-contained: takes FULL inputs, shards 8 samples/core (data parallel),
runs a Bass/Tile kernel via run_bass_kernel_spmd, gathers FULL output.

Math: edges are fully-connected per 64-node sample and e_ij = c_i - c_j,
so  z_ij @ W.T + b  separates into per-node terms:
    p_ij = alpha_i + beta_j      (sigmoid arg)
    q_ij = gamma_i + delta_j     (softplus arg)
    msg_ij  = sigmoid(p) * ln(1 + exp(q))
    agg_i   = sum_j msg_ij - msg_ii

Engine split per layer (per core: 8 samples, pairwise = 8x[128,64,64]):
  PE:  per-sample transposed projection tiles abT_s = [alphaT; betaT]
       ([64+64, 128] bf16, partition-offset matmuls of the x-part; the
       input-only c-parts+biases are HOST-precomputed and folded in by
       the PSUM drain), then "selector matmuls" abT_s @ Sel put
       P1(s) = alpha_i+beta_j directly into PSUM (Sel is a constant 0/1
       bf16 [128, 4096] matrix, K=128: rows 0-63 spread alphaT over j,
       rows 64-127 spread betaT over i). All projection matmuls run in
       bf16 (xT/x1b and the x-side weights are shipped/copied as bf16).
  ACT: sigmoid(P1) straight from 4-bank PSUM spans -> S bf16; softplus
       side via exp-factorization: two node-level exps, then ln(u+1.0)
       on the pairwise product; BN rstd; Square-accum for sumsq. Table
       loads amortized by 4-sample blocks (sigmoid-set alternating with
       ln-exp-set, ~4-5 loads/layer); the exps are emitted right after
       the previous BN so they reuse its ln-exp table.
  DVE: pairwise u = eg_i*ed_j outer mult in bf16 -- the pair-duplicated
       eg2 (eg2[f,2n]=eg2[f,2n+1]=eg[f,n]) keeps every operand
       innermost-packed 2-byte so TENSOR_TENSOR hits the 2x_1p
       2-elem/cycle mode; fused multiply+centered-prefix-scan
       (AGNN_MULT_CSCAN, K from sample-0 diagonal msgs; full-prefix
       output, strided segment diffs); diagonal (self-edge) subtraction;
       PSUM drains; fused BN-apply+residual+relu (AGNN_BN_RES).
       Custom DVE ops have NO fast modes (always 1 elem/cycle) and the
       Pool/gpsimd engine is useless for offload: broadcast/strided
       tensor ops run ~7x below the cost model on Q7 and anything queued
       behind a collective doorbell stalls until the collective retires.
  GPSIMD: only collective doorbells.
BN is global: one AllGather of [sum, sumsq] per 4-sample block (2 per
layer) -- the first block's collective doubles as the firmware warmup
for the layer and hides its latency under the second block's compute; a
dependency-free warmup AllGather at kernel start (ordered before the
first BN collective) absorbs the ~35us cold-firmware cost.

Schedule notes: Tile's static semaphore schedule follows its cost-model
sim; to make the B1 pipeline overlap real, the first B1 sample's
selector fills and ALL B1 u-mults are emitted before ln-block B0
(per-sample U tiles make that safe). Prefix tiles are fp16 (the scan's
internal accumulator stays fp32; only the write rounds). T tiles
rotate mod 3, which lets ln1p(s4)/ln1p(s5)/ln1p(s6) be emitted at the
END of ln-block B0 (T is independent of S, and each one's T-tile WAR
predecessor scan(s-3) is already emitted) so their scans run during
the B1 sigma phase; the layer tail keeps only sample 7, which is
itself half-pipelined (ln(7,h0) -> scan(7,h0) runs under ln(7,h1);
per-half prefix blocks with their own zero columns in pref7).
CAUTION: emitting lns batched before their aliased scans corrupts
T-tile versioning and hangs real HW even though the Tile sim passes -
keep per-sample ln->scan interleave and only append lns whose aliased
predecessors are already emitted.

Measured: ~230-246 ns*1e3 HW exec across runs (median ~235us, best
230.4us - the mod-3 tail trick also collapsed the variance band) vs
301.6us baseline; rel err ~5.1e-3 (gate 2e-2). NOTE: making x1 bf16
end-to-end (dropping the x1b copy) regressed to 465us once - the
BNRES-bf16-out schedule is toxic; keep x1 fp32 + separate x1b copy. Remaining costs: ACT
sigmoid+ln1p (~60us/layer floor at 1 elem/cycle), DVE scans
(~35us/layer, custom ops have no fast modes), last-block collective
tail (~12-15us/layer; raw remote-DMA is not available under the axon
tunnel - fake libnrt has no routing map), startup DMA ramp ~12us.
"""

import numpy as np

N_SAMPLES = 64
N_AGENTS = 64
N = N_SAMPLES * N_AGENTS          # 4096
F = 128
EDIM = 2
BN_EPS = 1e-5
N_CORES = 8
S_PC = N_SAMPLES // N_CORES       # 8 samples per core
NODES_PC = S_PC * N_AGENTS        # 512 nodes per core
NPAIR = N_AGENTS * N_AGENTS       # 4096 pairs per sample
BLK = 4                           # samples per ACT table-set block

# which sample-halves' u-mults run on gpsimd (Pool) instead of DVE.
# (s, h) pairs; tune for DVE/Pool balance.
POOL_MULT = set()

_CACHE = {}


def _register_custom_ops():
    import numpy as _np
    from concourse import dve_ops as D

    if getattr(D, "_agnn_ops", None):
        return D._agnn_ops
    from concourse.dve_spec import Spec, Src0, Src1, C0, C1, AluOp, scan, lower
    from concourse.dve_uop import DveOpSpec
    from concourse.dve_spec import relu as dve_relu

    def ref_mult_scan(in0, in1, s0, s1, imm2):
        prod = (in0.astype(_np.float32) * in1 - s0).astype(_np.float32)
        return _np.cumsum(prod.reshape(prod.shape[0], -1), 1).astype(
            _np.float32).reshape(in0.shape)

    def ref_diff_add(in0, in1, s0, s1, imm2):
        return (in0.astype(_np.float32) - in1 + s0).astype(_np.float32)

    def ref_bn_res(in0, in1, s0, s1, imm2):
        return _np.maximum(in0.astype(_np.float32) * s0 - s1 + in1, 0.0).astype(
            _np.float32)

    def make(name, spec, subdim):
        row = D._CUSTOM_DVE_ROW_BASE + len(D.OPS)
        D._SUB_OPCODE_FOR_NAME[name] = row
        shas = {}
        for ver in ("v3", "v4"):
            u = lower(spec, ver=ver)
            shas[ver] = DveOpSpec(name=name, opcode=row, uops=u, rd1_en=True).sha(ver)
        op = D.DveOp(name, spec, subdim=subdim, uops_sha=shas)
        D.OPS.append(op)
        D.CUSTOM_DVE_SPECS[name] = spec
        return op

    sc = Spec(body=scan(AluOp.ADD, Src0 * Src1 - C0), reference=ref_mult_scan)
    df = Spec(body=Src0 - Src1 + C0, reference=ref_diff_add)
    br = Spec(body=dve_relu(Src0 * C0 - C1 + Src1), reference=ref_bn_res)
    D._agnn_ops = (make("AGNN_MULT_CSCAN", sc, False),
                   make("AGNN_DIFF_ADD", df, False),
                   make("AGNN_BN_RES", br, False))
    return D._agnn_ops


def _patch_act_tables():
    """Pin exp/ln to natural_log_exp_and_others so the table-load inserter
    doesn't thrash between exp_and_others and natural_log."""
    from concourse import bacc, mybir, hw_specs

    if getattr(bacc, "_act_tables_patched", False):
        return
    AF = mybir.ActivationFunctionType
    orig = hw_specs.get_activation_tables

    def patched(arch):
        t = orig(arch)
        out = {}
        for name, s in t.items():
            s = set(s)
            if name == "exp_and_others":
                s.discard(AF.Exp)
            if name == "natural_log":
                s.discard(AF.Ln)
            out[name] = s
        return out

    bacc.get_activation_tables = patched
    bacc._act_tables_patched = True


def _build_nc():
    from concourse import bacc, mybir
    from concourse.tile import TileContext
    from concourse.tile_rust import add_dep_helper

    _patch_act_tables()
    OP_SCAN, OP_DIFF, OP_BNRES = _register_custom_ops()

    f32 = mybir.dt.float32
    bf16 = mybir.dt.bfloat16
    fp16 = mybir.dt.float16
    AF = mybir.ActivationFunctionType
    OP = mybir.AluOpType
    AX = mybir.AxisListType

    nc = bacc.Bacc(trn_type="TRN2", target_bir_lowering=False, debug=False,
                   num_devices=N_CORES)

    xT = nc.declare_dram_parameter("xT", [F, NODES_PC], bf16, isOutput=False)
    SelD = nc.declare_dram_parameter("Sel", [F, NPAIR], bf16, isOutput=False)
    params = {}
    for l in (1, 2):
        for n in ("WaT", "WbT", "VaT", "VbT"):
            params[f"{n}{l}"] = nc.declare_dram_parameter(f"{n}{l}", [F, F], bf16,
                                                          isOutput=False)
        params[f"cpAB{l}"] = nc.declare_dram_parameter(
            f"cpAB{l}", [F, S_PC * F], bf16, isOutput=False)
        params[f"cpGD{l}"] = nc.declare_dram_parameter(
            f"cpGD{l}", [F, 2 * NODES_PC], bf16, isOutput=False)
        params[f"g{l}"] = nc.declare_dram_parameter(f"g{l}", [F, 1], f32, isOutput=False)
        params[f"be{l}"] = nc.declare_dram_parameter(f"be{l}", [F, 1], f32, isOutput=False)
    yT = nc.declare_dram_parameter("yT", [F, NODES_PC], f32, isOutput=True)

    cc_warm_in = nc.dram_tensor("cc_warm_in", [1, 2], f32)
    cc_warm_out = nc.dram_tensor("cc_warm_out", [N_CORES, 2], f32, addr_space="Shared")
    cc_in = {(l, b): nc.dram_tensor(f"cc_in{l}_{b}", [F, 2], f32)
             for l in (1, 2) for b in (0, 1)}
    cc_out = {(l, b): nc.dram_tensor(f"cc_out{l}_{b}", [N_CORES * F, 2], f32,
                                     addr_space="Shared")
              for l in (1, 2) for b in (0, 1)}

    with TileContext(nc) as tc:
        from contextlib import ExitStack
        with ExitStack() as ctx:
            io = ctx.enter_context(tc.tile_pool(name="io", bufs=1))
            wp = ctx.enter_context(tc.tile_pool(name="wp", bufs=1))
            node = ctx.enter_context(tc.tile_pool(name="node", bufs=1))
            pair = ctx.enter_context(tc.tile_pool(name="pair", bufs=1))
            psum = ctx.enter_context(tc.tile_pool(name="psum", bufs=1, space="PSUM"))
            small = ctx.enter_context(tc.tile_pool(name="small", bufs=1))

            # ---- load inputs & weights ----
            xt = io.tile([F, NODES_PC], bf16, tag="xt")
            nc.sync.dma_start(xt[:], xT.ap()[:, :])
            wt = {}
            sel = io.tile([F, NPAIR], bf16, tag="sel")
            for l in (1, 2):
                if l == 2:
                    nc.sync.dma_start(sel[:], SelD.ap()[:, :])
                # gamma/delta path first: the exps are the first ACT consumers
                for n in ("VaT", "VbT"):
                    t = wp.tile([F, F], bf16, tag=f"{n}{l}")
                    nc.sync.dma_start(t[:], params[f"{n}{l}"].ap()[:, :])
                    wt[f"{n}{l}"] = t
                t = wp.tile([F, 2 * NODES_PC], bf16, tag=f"cpGD{l}")
                nc.sync.dma_start(t[:], params[f"cpGD{l}"].ap()[:, :])
                wt[f"cpGD{l}"] = t
                for n in ("WaT", "WbT"):
                    t = wp.tile([F, F], bf16, tag=f"{n}{l}")
                    nc.sync.dma_start(t[:], params[f"{n}{l}"].ap()[:, :])
                    wt[f"{n}{l}"] = t
                t = wp.tile([F, S_PC * F], bf16, tag=f"cpAB{l}")
                nc.sync.dma_start(t[:], params[f"cpAB{l}"].ap()[:, :])
                wt[f"cpAB{l}"] = t
                for n in ("g", "be"):
                    t = wp.tile([F, 1], f32, tag=f"{n}{l}")
                    nc.sync.dma_start(t[:], params[f"{n}{l}"].ap()[:, :])
                    wt[f"{n}{l}"] = t

            eps_t = small.tile([F, 1], f32, tag="eps")
            nc.vector.memset(eps_t[:], BN_EPS)

            warm_ar = nc.gpsimd.collective_compute(
                "AllGather", mybir.AluOpType.bypass,
                replica_groups=[list(range(N_CORES))],
                ins=[cc_warm_in.ap().opt()], outs=[cc_warm_out.ap().opt()])


            act_chain = []

            def act(*args, **kw):
                i = nc.scalar.activation(*args, **kw)
                if act_chain:
                    add_dep_helper(i.ins, act_chain[-1].ins,
                                   reason="act set grouping")
                act_chain.append(i)
                return i

            def layer(l, x_in, x_out, x_mm):
                psA = psum.tile([F, 2048], f32, tag="psA")
                psB = psum.tile([F, 2048], f32, tag="psB")

                # ---- gamma/delta node projections (j/exp side) ----
                # gamma -> bank slice psA[:, :512]; delta -> psA[:, 512:1024]
                cpab = wt[f"cpAB{l}"]
                cpgd = wt[f"cpGD{l}"]
                ps_g = psA[:, 0:512]
                nc.tensor.matmul(ps_g, wt[f"VaT{l}"][:], x_mm[:], start=True, stop=True)
                ps_d = psA[:, 512:1024]
                nc.tensor.matmul(ps_d, wt[f"VbT{l}"][:], x_mm[:], start=True, stop=True)

                # gamma/delta leave PSUM with the c-part folded in by the
                # drain (exps later read the SBUF copies, after the sigma
                # phase has recycled psA)
                gsb = node.tile([F, NODES_PC], f32, tag="gsb")
                nc.vector.tensor_tensor(gsb[:], ps_g, cpgd[:][:, 0:NODES_PC],
                                        op=OP.add)
                dsb = node.tile([F, NODES_PC], f32, tag="dsb")
                nc.vector.tensor_tensor(dsb[:], ps_d, cpgd[:][:, NODES_PC:],
                                        op=OP.add)

                eg = node.tile([F, NODES_PC], bf16, tag="eg")
                ed = node.tile([F, NODES_PC], bf16, tag="ed")
                eg2 = node.tile([F, 2 * NODES_PC], bf16, tag="eg2")
                dup_eng = nc.vector

                def mk_exp():
                    act(eg[:], gsb[:], AF.Exp)
                    act(ed[:], dsb[:], AF.Exp)
                    # pair-duplicated eg2[f, 2n] = eg2[f, 2n+1] = eg[f, n]
                    dup_eng.tensor_scalar(
                        eg2[:].rearrange("p (n d) -> p n d", d=2),
                        eg[:].rearrange("p (n o) -> p n o", o=1).broadcast_to(
                            [F, NODES_PC, 2]),
                        1.0, None, op0=OP.mult)

                # ---- alpha/beta transposed projection tiles (PE) ----
                abT = {}
                for s in range(S_PC):
                    psP = psB[:, (s % 2) * 512: (s % 2) * 512 + F]
                    xs = x_mm[:][:, s * 64:(s + 1) * 64]
                    nc.tensor.matmul(psP[0:64, :], xs, wt[f"WaT{l}"][:],
                                     start=True, stop=True)
                    nc.tensor.matmul(psP[64:128, :], xs, wt[f"WbT{l}"][:],
                                     start=True, stop=True)
                    t = node.tile([F, F], bf16, tag=f"abT{s}")
                    nc.vector.tensor_tensor(t[:], psP,
                                            cpab[:][:, s * F:(s + 1) * F],
                                            op=OP.add)
                    abT[s] = t

                agg = node.tile([F, NODES_PC], f32, tag="agg")
                kt = small.tile([F, 2], f32, tag="kt")


                Ss = {}
                Us = {}

                def fill_half(s, si, h):
                    ps = (psA, psB)[(2 * si + h) % 2]
                    for k in range(4):
                        nc.tensor.matmul(
                            ps[:, k * 512:(k + 1) * 512], abT[s][:],
                            sel[:][:, h * 2048 + k * 512: h * 2048 + (k + 1) * 512],
                            start=True, stop=True)
                    return ps

                def sigma_block(b, prefilled=()):
                    # PE fills psA/psB halves, ACT sigmoids them into S tiles
                    for si in range(BLK):
                        s = b * BLK + si
                        St = pair.tile([F, NPAIR], bf16, tag=f"S{s % 4}")
                        Ss[s] = St
                        for h in (0, 1):
                            if (s, h) in prefilled:
                                ps = (psA, psB)[(2 * si + h) % 2]
                            else:
                                ps = fill_half(s, si, h)
                            act(St[:, h * 2048:(h + 1) * 2048], ps[:, 0:2048],
                                AF.Sigmoid)

                def mults_block(b):
                    # u = eg_i * ed_j, bf16 packed-pair trick -> 2x DVE mode
                    for si in range(BLK):
                        s = b * BLK + si
                        Ut = pair.tile([F, NPAIR], bf16, tag=f"U{s}")
                        Us[s] = Ut
                        for h in (0, 1):
                            out = Ut[:, h * 2048:(h + 1) * 2048].rearrange(
                                "p (i j32 jp) -> p i j32 jp", i=32, j32=32, jp=2)
                            base = s * 64 + h * 32
                            in0 = eg2[:, 2 * base: 2 * (base + 32)].rearrange(
                                "p (i o jp) -> p i o jp", i=32, o=1, jp=2
                            ).broadcast_to([F, 32, 32, 2])
                            in1 = ed[:, s * 64:(s + 1) * 64].rearrange(
                                "p (o j32 jp) -> p o j32 jp", o=1, j32=32, jp=2
                            ).broadcast_to([F, 32, 32, 2])
                            eng = nc.gpsimd if (l == 2 and (s, h + 1) in POOL_MULT) else nc.vector
                            eng.tensor_tensor(out, in0, in1, op=OP.mult)

                Ts = {}

                def ln_block(samples):
                    for s in samples:
                        Ut = Us[s]
                        Tt = pair.tile([F, NPAIR], bf16, tag=f"T{s % 3}")
                        Ts[s] = Tt
                        act(Tt[:], Ut[:], AF.Ln, bias=1.0)
                        if s == 0:
                            # centering constant K from sample-0 diagonal msgs
                            St = Ss[0]
                            dg = small.tile([F, N_AGENTS], f32, tag="dg")
                            nc.vector.tensor_tensor(
                                dg[:], St[:][:, 0:NPAIR:65], Tt[:][:, 0:NPAIR:65],
                                op=OP.mult)
                            nc.vector.tensor_reduce(kt[:, 0:1], dg[:], axis=AX.X,
                                                    op=OP.add)
                            nc.vector.tensor_scalar(kt[:, 0:1], kt[:, 0:1],
                                                    1.0 / N_AGENTS, None, op0=OP.mult)
                            nc.vector.tensor_scalar(kt[:, 1:2], kt[:, 0:1],
                                                    float(N_AGENTS), None, op0=OP.mult)

                def scan_block(samples):
                    for s in samples:
                        St, Tt = Ss[s], Ts[s]
                        pref = pair.tile([F, 1 + NPAIR], fp16, tag=f"pref{s % 2}")
                        nc.vector.memset(pref[:, 0:1], 0.0)
                        nc.vector._custom_dve(OP_SCAN, out=pref[:, 1:1 + NPAIR],
                                              in0=St[:], in1=Tt[:], s0=kt[:, 0:1])
                        sl = slice(s * 64, (s + 1) * 64)
                        ends = pref[:, 1:1 + NPAIR].rearrange(
                            "p (i j) -> p i j", j=N_AGENTS)[:, :, N_AGENTS - 1:N_AGENTS]
                        prevs = pref[:, 0:NPAIR].rearrange(
                            "p (i j) -> p i j", j=N_AGENTS)[:, :, 0:1]
                        nc.vector._custom_dve(
                            OP_DIFF,
                            out=agg[:, sl].rearrange("p (i o) -> p i o", o=1),
                            in0=ends, in1=prevs, s0=kt[:, 1:2])
                        # subtract self-edge (diagonal) messages
                        dtmp = small.tile([F, N_AGENTS], f32, tag=f"dt{s % 2}")
                        nc.vector.tensor_tensor(dtmp[:], St[:][:, 0:NPAIR:65],
                                                Tt[:][:, 0:NPAIR:65], op=OP.mult)
                        nc.vector.tensor_tensor(agg[:, sl], agg[:, sl], dtmp[:],
                                                op=OP.subtract)

                gaths = {}

                def block_stats(b):
                    # partial BN stats over this block's 4 samples; the b=0
                    # collective doubles as the firmware warmup and hides its
                    # latency under the b=1 half of the layer
                    sl = slice(b * NODES_PC // 2, (b + 1) * NODES_PC // 2)
                    s2 = small.tile([F, 2], f32, tag=f"s2_{b}")
                    nc.vector.tensor_reduce(s2[:, 0:1], agg[:, sl], axis=AX.X,
                                            op=OP.add)
                    trash = node.tile([F, NODES_PC // 2], f32, tag=f"trash{b}")
                    act(trash[:], agg[:, sl], AF.Square, accum_out=s2[:, 1:2])
                    dsum = nc.sync.dma_start(cc_in[(l, b)].ap()[:, :], s2[:])
                    ar = nc.gpsimd.collective_compute(
                        "AllGather", mybir.AluOpType.bypass,
                        replica_groups=[list(range(N_CORES))],
                        ins=[cc_in[(l, b)].ap().opt()],
                        outs=[cc_out[(l, b)].ap().opt()])
                    add_dep_helper(ar.ins, dsum.ins, reason="cc reads cc_in")
                    if l == 1 and b == 0:
                        add_dep_helper(ar.ins, warm_ar.ins,
                                       reason="warmup before first bn cc")
                    gath = small.tile([F, 2, N_CORES], f32, tag=f"gath{b}")
                    din = nc.sync.dma_start(
                        gath[:],
                        cc_out[(l, b)].ap().rearrange("(r p) c -> p c r", r=N_CORES))
                    add_dep_helper(din.ins, ar.ins, reason="dma reads cc_out")
                    gaths[b] = gath

                def ln_scan(samples):
                    for s in samples:
                        ln_block([s])
                        scan_block([s])

                mk_exp()
                sigma_block(0)
                mults_block(0)
                fill_half(BLK, 0, 0)
                fill_half(BLK, 0, 1)
                mults_block(1)
                ln_scan([0, 1, 2, 3])
                ln_block([4])
                ln_block([5])
                ln_block([6])
                block_stats(0)
                sigma_block(1, prefilled={(BLK, 0), (BLK, 1)})
                scan_block([4])
                scan_block([5])
                scan_block([6])

                # tail sample 7: halves pipelined (ln h1 overlaps scan h0)
                # to shorten the serial chain into the last BN collective
                St7, Ut7 = Ss[7], Us[7]
                Tt7 = pair.tile([F, NPAIR], bf16, tag=f"T{7 % 3}")
                Ts[7] = Tt7
                pref7 = pair.tile([F, 2 + NPAIR], fp16, tag="pref7")
                nc.vector.memset(pref7[:, 0:1], 0.0)
                nc.vector.memset(pref7[:, 2049:2050], 0.0)
                HN = NPAIR // 2
                for h in (0, 1):
                    hsl = slice(h * HN, (h + 1) * HN)
                    act(Tt7[:, hsl], Ut7[:, hsl], AF.Ln, bias=1.0)
                    base = h * (HN + 1)
                    nc.vector._custom_dve(
                        OP_SCAN, out=pref7[:, base + 1:base + 1 + HN],
                        in0=St7[:, hsl], in1=Tt7[:, hsl], s0=kt[:, 0:1])
                    ends = pref7[:, base + 1:base + 1 + HN].rearrange(
                        "p (i j) -> p i j", j=N_AGENTS)[:, :, N_AGENTS - 1:N_AGENTS]
                    prevs = pref7[:, base:base + HN].rearrange(
                        "p (i j) -> p i j", j=N_AGENTS)[:, :, 0:1]
                    asl = slice(7 * 64 + h * 32, 7 * 64 + (h + 1) * 32)
                    nc.vector._custom_dve(
                        OP_DIFF, out=agg[:, asl].rearrange("p (i o) -> p i o", o=1),
                        in0=ends, in1=prevs, s0=kt[:, 1:2])
                dtmp7 = small.tile([F, N_AGENTS], f32, tag="dt7")
                nc.vector.tensor_tensor(dtmp7[:], St7[:][:, 0:NPAIR:65],
                                        Tt7[:][:, 0:NPAIR:65], op=OP.mult)
                sl7 = slice(7 * 64, 8 * 64)
                nc.vector.tensor_tensor(agg[:, sl7], agg[:, sl7], dtmp7[:],
                                        op=OP.subtract)
                block_stats(1)

                gst = small.tile([F, 2], f32, tag="gst")
                nc.vector.tensor_reduce(gst[:], gaths[0][:], axis=AX.X, op=OP.add)
                gs1 = small.tile([F, 2], f32, tag="gs1")
                nc.vector.tensor_reduce(gs1[:], gaths[1][:], axis=AX.X, op=OP.add)
                nc.vector.tensor_tensor(gst[:], gst[:], gs1[:], op=OP.add)

                # ---- BN apply + residual + relu ----
                me2 = small.tile([F, 2], f32, tag="me2")
                nc.vector.tensor_scalar(me2[:], gst[:], 1.0 / N, None, op0=OP.mult)
                mean, ex2 = me2[:, 0:1], me2[:, 1:2]
                var = small.tile([F, 1], f32, tag="var")
                nc.vector.tensor_tensor(var[:], mean, mean, op=OP.mult)
                nc.vector.tensor_tensor(var[:], ex2, var[:], op=OP.subtract)
                lnv = small.tile([F, 1], f32, tag="lnv")
                act(lnv[:], var[:], AF.Ln, bias=eps_t[:])
                rstd = small.tile([F, 1], f32, tag="rstd")
                act(rstd[:], lnv[:], AF.Exp, bias=0.0, scale=-0.5)
                scal = small.tile([F, 1], f32, tag="scal")
                nc.vector.tensor_tensor(scal[:], rstd[:], wt[f"g{l}"][:], op=OP.mult)
                shneg = small.tile([F, 1], f32, tag="shneg")
                nc.vector.tensor_scalar(shneg[:], mean, scal[:, 0:1],
                                        wt[f"be{l}"][:][:, 0:1],
                                        op0=OP.mult, op1=OP.subtract)
                nc.vector._custom_dve(OP_BNRES, out=x_out[:], in0=agg[:],
                                      in1=x_in[:], s0=scal[:, 0:1], s1=shneg[:, 0:1])

            x1 = io.tile([F, NODES_PC], f32, tag="x1")
            layer(1, xt, x1, xt)
            x1b = io.tile([F, NODES_PC], bf16, tag="x1b")
            nc.vector.tensor_scalar(x1b[:], x1[:], 1.0, None, op0=OP.mult)
            x2 = io.tile([F, NODES_PC], f32, tag="x2")
            layer(2, x1, x2, x1b)
            nc.sync.dma_start(yT.ap()[:, :], x2[:])

    nc.compile()
    return nc


def _get_nc():
    if "nc" not in _CACHE:
        _CACHE["nc"] = _build_nc()
    return _CACHE["nc"]


def _build_sel():
    """Sel[k, i*64+j] = (k<64 and k==i) or (k>=64 and k-64==j), bf16."""
    import ml_dtypes
    S = np.zeros((F, NPAIR), np.float32)
    for i in range(N_AGENTS):
        S[i, i * N_AGENTS:(i + 1) * N_AGENTS] = 1.0
    for j in range(N_AGENTS):
        S[64 + j, j::N_AGENTS] = 1.0
    return S.astype(ml_dtypes.bfloat16)


def _canonical_edge_ok(src, dst):
    idx = np.arange(N_AGENTS)
    rows = np.repeat(idx, N_AGENTS)
    cols = np.tile(idx, N_AGENTS)
    m = rows != cols
    rows, cols = rows[m], cols[m]
    offs = (np.arange(N_SAMPLES) * N_AGENTS)[:, None]
    csrc = (rows[None, :] + offs).ravel().astype(np.int64)
    cdst = (cols[None, :] + offs).ravel().astype(np.int64)
    if src.shape != csrc.shape:
        return False
    key = np.sort(src.astype(np.int64) * N + dst.astype(np.int64))
    ckey = np.sort(csrc * N + cdst)
    return bool(np.array_equal(key, ckey))


def _numpy_fallback(gnn_in, centers, src, dst, Ws_all):
    def sig(x):
        return 1.0 / (1.0 + np.exp(-x))

    def sp(x):
        return np.log1p(np.exp(-np.abs(x))) + np.maximum(x, 0.0)

    x = gnn_in.astype(np.float64)
    e = (centers[dst] - centers[src]).astype(np.float64)
    for (Wf, bf, Wsm, bs, g, be) in Ws_all:
        z = np.concatenate([x[dst], x[src], e], axis=-1)
        msg = sig(z @ Wf.T + bf) * sp(z @ Wsm.T + bs)
        agg = np.zeros_like(x)
        np.add.at(agg, dst, msg)
        mean = agg.mean(0)
        var = agg.var(0)
        agg = (agg - mean) / np.sqrt(var + BN_EPS) * g + be
        x = np.maximum(agg + x, 0.0)
    return x.astype(np.float32)


def _host_weights(Wf, bf, Ws, bs):
    """lhsT forms for the projection matmuls.

    WcAll layout [3, 4F]: cols 0:F   = c-part+bias of alpha (Wc3a)
                          cols F:2F  = c-part of beta (Wc3b)
                          cols 2F:3F = c-part+bias of gamma (Vc3g)
                          cols 3F:4F = c-part of delta (Vc3d)
    """
    WaT = np.ascontiguousarray(Wf[:, :F].T)
    WbT = np.ascontiguousarray(Wf[:, F:2 * F].T)
    Wc = Wf[:, 2 * F:2 * F + EDIM].T           # [2, 128]
    z = np.zeros((1, F), np.float32)
    Wc3a = np.concatenate([Wc, bf[None, :]], 0)
    Wc3b = np.concatenate([-Wc, z], 0)
    VaT = np.ascontiguousarray(Ws[:, :F].T)
    VbT = np.ascontiguousarray(Ws[:, F:2 * F].T)
    Vc = Ws[:, 2 * F:2 * F + EDIM].T
    Vc3g = np.concatenate([Vc, bs[None, :]], 0)
    Vc3d = np.concatenate([-Vc, z], 0)
    return WaT, WbT, VaT, VbT, (Wc3a, Wc3b, Vc3g, Vc3d)


def kernel(gnn_in, centers, src, dst,
           Wf1, bf1, Ws1, bs1, g1, be1,
           Wf2, bf2, Ws2, bs2, g2, be2,
           _trace=False, _tmpdir=None):
    gnn_in = np.ascontiguousarray(np.asarray(gnn_in, np.float32))
    centers = np.ascontiguousarray(np.asarray(centers, np.float32))
    src = np.asarray(src, np.int32)
    dst = np.asarray(dst, np.int32)
    args = [np.asarray(a, np.float32) for a in
            (Wf1, bf1, Ws1, bs1, g1, be1, Wf2, bf2, Ws2, bs2, g2, be2)]
    (Wf1, bf1, Ws1, bs1, g1, be1, Wf2, bf2, Ws2, bs2, g2, be2) = args

    if not _canonical_edge_ok(src, dst):
        import sys
        print("kernel.py: edge index is not block-fully-connected; numpy fallback",
              file=sys.stderr)
        return _numpy_fallback(gnn_in, centers, src, dst,
                               [(Wf1, bf1, Ws1, bs1, g1, be1),
                                (Wf2, bf2, Ws2, bs2, g2, be2)])

    import ml_dtypes
    from concourse import bass_utils

    nc = _get_nc()

    w1 = _host_weights(Wf1, bf1, Ws1, bs1)
    w2 = _host_weights(Wf2, bf2, Ws2, bs2)
    wmap = {}
    wc3 = {}
    for l, w in ((1, w1), (2, w2)):
        for n, a in zip(("WaT", "WbT", "VaT", "VbT"), w[:4]):
            wmap[f"{n}{l}"] = a.astype(ml_dtypes.bfloat16)
        wc3[l] = w[4]
    wmap["g1"] = np.ascontiguousarray(g1[:, None])
    wmap["be1"] = np.ascontiguousarray(be1[:, None])
    wmap["g2"] = np.ascontiguousarray(g2[:, None])
    wmap["be2"] = np.ascontiguousarray(be2[:, None])
    wmap["Sel"] = _build_sel()

    in_maps = []
    for k in range(N_CORES):
        sl = slice(k * NODES_PC, (k + 1) * NODES_PC)
        m = dict(wmap)
        m["xT"] = np.ascontiguousarray(gnn_in[sl].T).astype(ml_dtypes.bfloat16)
        c3k = np.concatenate([centers[sl].T, np.ones((1, NODES_PC), np.float32)], 0)
        for l in (1, 2):
            Wc3a, Wc3b, Vc3g, Vc3d = wc3[l]
            blocks = []
            for s in range(S_PC):
                cs = c3k[:, s * 64:(s + 1) * 64]             # [3, 64]
                blocks.append(np.concatenate([cs.T @ Wc3a, cs.T @ Wc3b], 0))
            m[f"cpAB{l}"] = np.ascontiguousarray(
                np.concatenate(blocks, 1).astype(ml_dtypes.bfloat16))
            m[f"cpGD{l}"] = np.ascontiguousarray(np.concatenate(
                [Vc3g.T @ c3k, Vc3d.T @ c3k], 1).astype(ml_dtypes.bfloat16))
        in_maps.append(m)

    kw = {}
    if _trace:
        kw = dict(trace=True, tmpdir=_tmpdir)
    res = bass_utils.run_bass_kernel_spmd(nc, in_maps, core_ids=list(range(N_CORES)), **kw)

    out = np.empty((N, F), np.float32)
    for k in range(N_CORES):
        out[k * NODES_PC:(k + 1) * NODES_PC] = res.results[k]["yT"].T
    if _trace:
        _CACHE["last_res"] = res
    return out


# revision 44
# speedup vs baseline: 1.0340x; 1.0340x over previous
"""AgentGNN (2x CGConv + BN + residual + ReLU) on 8 TRN2 NeuronCores.

Self-contained: takes FULL inputs, shards 8 samples/core (data parallel),
runs a Bass/Tile kernel via run_bass_kernel_spmd, gathers FULL output.

Math: edges are fully-connected per 64-node sample and e_ij = c_i - c_j,
so  z_ij @ W.T + b  separates into per-node terms:
    p_ij = alpha_i + beta_j      (sigmoid arg)
    q_ij = gamma_i + delta_j     (softplus arg)
    msg_ij  = sigmoid(p) * ln(1 + exp(q))
    agg_i   = sum_j msg_ij - msg_ii

Engine split per layer (per core: 8 samples, pairwise = 8x[128,64,64]):
  PE:  per-sample transposed projection tiles abT_s = [alphaT; betaT]
       ([64+64, 128] bf16, partition-offset matmuls of the x-part; the
       input-only c-parts+biases are HOST-precomputed and folded in by
       the PSUM drain), then "selector matmuls" abT_s @ Sel put
       P1(s) = alpha_i+beta_j directly into PSUM (Sel is a constant 0/1
       bf16 [128, 4096] matrix, K=128: rows 0-63 spread alphaT over j,
       rows 64-127 spread betaT over i). All projection matmuls run in
       bf16 (xT/x1b and the x-side weights are shipped/copied as bf16).
  ACT: sigmoid(P1) straight from 4-bank PSUM spans -> S bf16; softplus
       side via exp-factorization: two node-level exps, then ln(u+1.0)
       on the pairwise product; BN rstd; Square-accum for sumsq. Table
       loads amortized by 4-sample blocks (sigmoid-set alternating with
       ln-exp-set, ~4-5 loads/layer); the exps are emitted right after
       the previous BN so they reuse its ln-exp table.
  DVE: pairwise u = eg_i*ed_j outer mult in bf16 -- the pair-duplicated
       eg2 (eg2[f,2n]=eg2[f,2n+1]=eg[f,n]) keeps every operand
       innermost-packed 2-byte so TENSOR_TENSOR hits the 2x_1p
       2-elem/cycle mode; fused multiply+centered-prefix-scan
       (AGNN_MULT_CSCAN, K from sample-0 diagonal msgs; full-prefix
       output, strided segment diffs); diagonal (self-edge) subtraction;
       PSUM drains; fused BN-apply+residual+relu (AGNN_BN_RES).
       Custom DVE ops have NO fast modes (always 1 elem/cycle) and the
       Pool/gpsimd engine is useless for offload: broadcast/strided
       tensor ops run ~7x below the cost model on Q7 and anything queued
       behind a collective doorbell stalls until the collective retires.
  GPSIMD: only collective doorbells.
BN is global: one AllGather of [sum, sumsq] per 4-sample block (2 per
layer) -- the first block's collective doubles as the firmware warmup
for the layer and hides its latency under the second block's compute; a
dependency-free warmup AllGather at kernel start (ordered before the
first BN collective) absorbs the ~35us cold-firmware cost.

Schedule notes: Tile's static semaphore schedule follows its cost-model
sim; to make the B1 pipeline overlap real, the first B1 sample's
selector fills and ALL B1 u-mults are emitted before ln-block B0
(per-sample U tiles make that safe). Prefix tiles are fp16 (the scan's
internal accumulator stays fp32; only the write rounds). T tiles
rotate mod 3, which lets ln1p(s4)/ln1p(s5)/ln1p(s6) be emitted at the
END of ln-block B0 (T is independent of S, and each one's T-tile WAR
predecessor scan(s-3) is already emitted) so their scans run during
the B1 sigma phase; the layer tail keeps only sample 7, which is
itself half-pipelined (ln(7,h0) -> scan(7,h0) runs under ln(7,h1);
per-half prefix blocks with their own zero columns in pref7).
CAUTION: emitting lns batched before their aliased scans corrupts
T-tile versioning and hangs real HW even though the Tile sim passes -
keep per-sample ln->scan interleave and only append lns whose aliased
predecessors are already emitted.

Measured: ~230-246 ns*1e3 HW exec across runs (median ~235us, best
230.4us - the mod-3 tail trick also collapsed the variance band) vs
301.6us baseline; rel err ~5.1e-3 (gate 2e-2). NOTE: making x1 bf16
end-to-end (dropping the x1b copy) regressed to 465us once - the
BNRES-bf16-out schedule is toxic; keep x1 fp32 + separate x1b copy. Remaining costs: ACT
sigmoid+ln1p (~60us/layer floor at 1 elem/cycle), DVE scans
(~35us/layer, custom ops have no fast modes), last-block collective
tail (~12-15us/layer; raw remote-DMA is not available under the axon
tunnel - fake libnrt has no routing map), startup DMA ramp ~12us.
"""

import numpy as np

N_SAMPLES = 64
N_AGENTS = 64
N = N_SAMPLES * N_AGENTS          # 4096
F = 128
EDIM = 2
BN_EPS = 1e-5
N_CORES = 8
S_PC = N_SAMPLES // N_CORES       # 8 samples per core
NODES_PC = S_PC * N_AGENTS        # 512 nodes per core
NPAIR = N_AGENTS * N_AGENTS       # 4096 pairs per sample
BLK = 4                           # samples per ACT table-set block

# which sample-halves' u-mults run on gpsimd (Pool) instead of DVE.
# (s, h) pairs; tune for DVE/Pool balance.
POOL_MULT = set()

_CACHE = {}


def _register_custom_ops():
    import numpy as _np
    from concourse import dve_ops as D

    if getattr(D, "_agnn_ops", None):
        return D._agnn_ops
    from concourse.dve_spec import Spec, Src0, Src1, C0, C1, AluOp, scan, lower
    from concourse.dve_uop import DveOpSpec
    from concourse.dve_spec import relu as dve_relu

    def ref_mult_scan(in0, in1, s0, s1, imm2):
        prod = (in0.astype(_np.float32) * in1 - s0).astype(_np.float32)
        return _np.cumsum(prod.reshape(prod.shape[0], -1), 1).astype(
            _np.float32).reshape(in0.shape)

    def ref_diff_add(in0, in1, s0, s1, imm2):
        return (in0.astype(_np.float32) - in1 + s0).astype(_np.float32)

    def ref_bn_res(in0, in1, s0, s1, imm2):
        return _np.maximum(in0.astype(_np.float32) * s0 - s1 + in1, 0.0).astype(
            _np.float32)

    def make(name, spec, subdim):
        row = D._CUSTOM_DVE_ROW_BASE + len(D.OPS)
        D._SUB_OPCODE_FOR_NAME[name] = row
        shas = {}
        for ver in ("v3", "v4"):
            u = lower(spec, ver=ver)
            shas[ver] = DveOpSpec(name=name, opcode=row, uops=u, rd1_en=True).sha(ver)
        op = D.DveOp(name, spec, subdim=subdim, uops_sha=shas)
        D.OPS.append(op)
        D.CUSTOM_DVE_SPECS[name] = spec
        return op

    sc = Spec(body=scan(AluOp.ADD, Src0 * Src1 - C0), reference=ref_mult_scan)
    df = Spec(body=Src0 - Src1 + C0, reference=ref_diff_add)
    br = Spec(body=dve_relu(Src0 * C0 - C1 + Src1), reference=ref_bn_res)
    D._agnn_ops = (make("AGNN_MULT_CSCAN", sc, False),
                   make("AGNN_DIFF_ADD", df, False),
                   make("AGNN_BN_RES", br, False))
    return D._agnn_ops


def _patch_act_tables():
    """Pin exp/ln to natural_log_exp_and_others so the table-load inserter
    doesn't thrash between exp_and_others and natural_log."""
    from concourse import bacc, mybir, hw_specs

    if getattr(bacc, "_act_tables_patched", False):
        return
    AF = mybir.ActivationFunctionType
    orig = hw_specs.get_activation_tables

    def patched(arch):
        t = orig(arch)
        out = {}
        for name, s in t.items():
            s = set(s)
            if name == "exp_and_others":
                s.discard(AF.Exp)
            if name == "natural_log":
                s.discard(AF.Ln)
            out[name] = s
        return out

    bacc.get_activation_tables = patched
    bacc._act_tables_patched = True


def _build_nc():
    from concourse import bacc, mybir
    from concourse.tile import TileContext
    from concourse.tile_rust import add_dep_helper

    _patch_act_tables()
    OP_SCAN, OP_DIFF, OP_BNRES = _register_custom_ops()

    f32 = mybir.dt.float32
    bf16 = mybir.dt.bfloat16
    fp16 = mybir.dt.float16
    AF = mybir.ActivationFunctionType
    OP = mybir.AluOpType
    AX = mybir.AxisListType

    nc = bacc.Bacc(trn_type="TRN2", target_bir_lowering=False, debug=False,
                   num_devices=N_CORES)

    xT = nc.declare_dram_parameter("xT", [F, NODES_PC], bf16, isOutput=False)
    SelD = nc.declare_dram_parameter("Sel", [F, NPAIR], bf16, isOutput=False)
    params = {}
    for l in (1, 2):
        # GDpack: [VaT | VbT | cpGD] - the gamma/delta (exp) critical path
        params[f"GD{l}"] = nc.declare_dram_parameter(
            f"GD{l}", [F, 2 * F + 2 * NODES_PC], bf16, isOutput=False)
        # ABpack: [WaT | WbT | cpAB]
        params[f"AB{l}"] = nc.declare_dram_parameter(
            f"AB{l}", [F, 2 * F + S_PC * F], bf16, isOutput=False)
    params["gbe"] = nc.declare_dram_parameter("gbe", [F, 4], f32, isOutput=False)
    yT = nc.declare_dram_parameter("yT", [F, NODES_PC], f32, isOutput=True)

    cc_warm_in = nc.dram_tensor("cc_warm_in", [1, 2], f32)
    cc_warm_out = nc.dram_tensor("cc_warm_out", [N_CORES, 2], f32, addr_space="Shared")
    cc_in = {(l, b): nc.dram_tensor(f"cc_in{l}_{b}", [F, 2], f32)
             for l in (1, 2) for b in (0, 1)}
    cc_out = {(l, b): nc.dram_tensor(f"cc_out{l}_{b}", [N_CORES * F, 2], f32,
                                     addr_space="Shared")
              for l in (1, 2) for b in (0, 1)}

    with TileContext(nc) as tc:
        from contextlib import ExitStack
        with ExitStack() as ctx:
            io = ctx.enter_context(tc.tile_pool(name="io", bufs=1))
            wp = ctx.enter_context(tc.tile_pool(name="wp", bufs=1))
            node = ctx.enter_context(tc.tile_pool(name="node", bufs=1))
            pair = ctx.enter_context(tc.tile_pool(name="pair", bufs=1))
            psum = ctx.enter_context(tc.tile_pool(name="psum", bufs=1, space="PSUM"))
            small = ctx.enter_context(tc.tile_pool(name="small", bufs=1))

            # ---- load inputs & weights ----
            xt = io.tile([F, NODES_PC], bf16, tag="xt")
            nc.sync.dma_start(xt[:], xT.ap()[:, :])
            wt = {}
            sel = io.tile([F, NPAIR], bf16, tag="sel")
            for l in (1, 2):
                if l == 2:
                    nc.sync.dma_start(sel[:], SelD.ap()[:, :])
                gdt = wp.tile([F, 2 * F + 2 * NODES_PC], bf16, tag=f"GD{l}")
                nc.sync.dma_start(gdt[:], params[f"GD{l}"].ap()[:, :])
                wt[f"VaT{l}"] = gdt[:][:, 0:F]
                wt[f"VbT{l}"] = gdt[:][:, F:2 * F]
                wt[f"cpGD{l}"] = gdt[:][:, 2 * F:]
                abt = wp.tile([F, 2 * F + S_PC * F], bf16, tag=f"AB{l}")
                nc.sync.dma_start(abt[:], params[f"AB{l}"].ap()[:, :])
                wt[f"WaT{l}"] = abt[:][:, 0:F]
                wt[f"WbT{l}"] = abt[:][:, F:2 * F]
                wt[f"cpAB{l}"] = abt[:][:, 2 * F:]
            gbe_t = wp.tile([F, 4], f32, tag="gbe")
            nc.sync.dma_start(gbe_t[:], params["gbe"].ap()[:, :])
            for l in (1, 2):
                wt[f"g{l}"] = gbe_t[:][:, 2 * (l - 1):2 * (l - 1) + 1]
                wt[f"be{l}"] = gbe_t[:][:, 2 * (l - 1) + 1:2 * (l - 1) + 2]

            eps_t = small.tile([F, 1], f32, tag="eps")
            nc.vector.memset(eps_t[:], BN_EPS)

            warm_ar = nc.gpsimd.collective_compute(
                "AllGather", mybir.AluOpType.bypass,
                replica_groups=[list(range(N_CORES))],
                ins=[cc_warm_in.ap().opt()], outs=[cc_warm_out.ap().opt()])


            act_chain = []

            def act(*args, **kw):
                i = nc.scalar.activation(*args, **kw)
                if act_chain:
                    add_dep_helper(i.ins, act_chain[-1].ins,
                                   reason="act set grouping")
                act_chain.append(i)
                return i

            def layer(l, x_in, x_out, x_mm):
                psA = psum.tile([F, 2048], f32, tag="psA")
                psB = psum.tile([F, 2048], f32, tag="psB")

                # ---- gamma/delta node projections (j/exp side) ----
                # gamma -> bank slice psA[:, :512]; delta -> psA[:, 512:1024]
                cpab = wt[f"cpAB{l}"]
                cpgd = wt[f"cpGD{l}"]
                # (both are APs into the packed weight tiles)
                ps_g = psA[:, 0:512]
                nc.tensor.matmul(ps_g, wt[f"VaT{l}"], x_mm[:], start=True, stop=True)
                ps_d = psA[:, 512:1024]
                nc.tensor.matmul(ps_d, wt[f"VbT{l}"], x_mm[:], start=True, stop=True)

                # gamma/delta leave PSUM with the c-part folded in by the
                # drain (exps later read the SBUF copies, after the sigma
                # phase has recycled psA)
                gsb = node.tile([F, NODES_PC], f32, tag="gsb")
                nc.vector.tensor_tensor(gsb[:], ps_g, cpgd[:, 0:NODES_PC],
                                        op=OP.add)
                dsb = node.tile([F, NODES_PC], f32, tag="dsb")
                nc.vector.tensor_tensor(dsb[:], ps_d, cpgd[:, NODES_PC:],
                                        op=OP.add)

                eg = node.tile([F, NODES_PC], bf16, tag="eg")
                ed = node.tile([F, NODES_PC], bf16, tag="ed")
                eg2 = node.tile([F, 2 * NODES_PC], bf16, tag="eg2")
                dup_eng = nc.vector

                def mk_exp():
                    act(eg[:], gsb[:], AF.Exp)
                    act(ed[:], dsb[:], AF.Exp)
                    # pair-duplicated eg2[f, 2n] = eg2[f, 2n+1] = eg[f, n]
                    dup_eng.tensor_scalar(
                        eg2[:].rearrange("p (n d) -> p n d", d=2),
                        eg[:].rearrange("p (n o) -> p n o", o=1).broadcast_to(
                            [F, NODES_PC, 2]),
                        1.0, None, op0=OP.mult)

                # ---- alpha/beta transposed projection tiles (PE) ----
                abT = {}
                for s in range(S_PC):
                    psP = psB[:, (s % 2) * 512: (s % 2) * 512 + F]
                    xs = x_mm[:][:, s * 64:(s + 1) * 64]
                    nc.tensor.matmul(psP[0:64, :], xs, wt[f"WaT{l}"],
                                     start=True, stop=True)
                    nc.tensor.matmul(psP[64:128, :], xs, wt[f"WbT{l}"],
                                     start=True, stop=True)
                    t = node.tile([F, F], bf16, tag=f"abT{s}")
                    nc.vector.tensor_tensor(t[:], psP,
                                            cpab[:, s * F:(s + 1) * F],
                                            op=OP.add)
                    abT[s] = t

                agg = node.tile([F, NODES_PC], f32, tag="agg")
                kt = small.tile([F, 2], f32, tag="kt")


                Ss = {}
                Us = {}

                def fill_half(s, si, h):
                    ps = (psA, psB)[(2 * si + h) % 2]
                    for k in range(4):
                        nc.tensor.matmul(
                            ps[:, k * 512:(k + 1) * 512], abT[s][:],
                            sel[:][:, h * 2048 + k * 512: h * 2048 + (k + 1) * 512],
                            start=True, stop=True)
                    return ps

                def sigma_block(b, prefilled=()):
                    # PE fills psA/psB halves, ACT sigmoids them into S tiles
                    for si in range(BLK):
                        s = b * BLK + si
                        St = pair.tile([F, NPAIR], bf16, tag=f"S{s % 4}")
                        Ss[s] = St
                        for h in (0, 1):
                            if (s, h) in prefilled:
                                ps = (psA, psB)[(2 * si + h) % 2]
                            else:
                                ps = fill_half(s, si, h)
                            act(St[:, h * 2048:(h + 1) * 2048], ps[:, 0:2048],
                                AF.Sigmoid)

                def mults_block(b):
                    # u = eg_i * ed_j, bf16 packed-pair trick -> 2x DVE mode
                    for si in range(BLK):
                        s = b * BLK + si
                        Ut = pair.tile([F, NPAIR], bf16, tag=f"U{s}")
                        Us[s] = Ut
                        for h in (0, 1):
                            out = Ut[:, h * 2048:(h + 1) * 2048].rearrange(
                                "p (i j32 jp) -> p i j32 jp", i=32, j32=32, jp=2)
                            base = s * 64 + h * 32
                            in0 = eg2[:, 2 * base: 2 * (base + 32)].rearrange(
                                "p (i o jp) -> p i o jp", i=32, o=1, jp=2
                            ).broadcast_to([F, 32, 32, 2])
                            in1 = ed[:, s * 64:(s + 1) * 64].rearrange(
                                "p (o j32 jp) -> p o j32 jp", o=1, j32=32, jp=2
                            ).broadcast_to([F, 32, 32, 2])
                            eng = nc.gpsimd if (l == 2 and (s, h + 1) in POOL_MULT) else nc.vector
                            eng.tensor_tensor(out, in0, in1, op=OP.mult)

                Ts = {}

                def ln_block(samples):
                    for s in samples:
                        Ut = Us[s]
                        Tt = pair.tile([F, NPAIR], bf16, tag=f"T{s % 3}")
                        Ts[s] = Tt
                        act(Tt[:], Ut[:], AF.Ln, bias=1.0)
                        if s == 0:
                            # centering constant K from sample-0 diagonal msgs
                            St = Ss[0]
                            dg = small.tile([F, N_AGENTS], f32, tag="dg")
                            nc.vector.tensor_tensor(
                                dg[:], St[:][:, 0:NPAIR:65], Tt[:][:, 0:NPAIR:65],
                                op=OP.mult)
                            nc.vector.tensor_reduce(kt[:, 0:1], dg[:], axis=AX.X,
                                                    op=OP.add)
                            nc.vector.tensor_scalar(kt[:, 0:1], kt[:, 0:1],
                                                    1.0 / N_AGENTS, None, op0=OP.mult)
                            nc.vector.tensor_scalar(kt[:, 1:2], kt[:, 0:1],
                                                    float(N_AGENTS), None, op0=OP.mult)

                def scan_block(samples):
                    for s in samples:
                        St, Tt = Ss[s], Ts[s]
                        pref = pair.tile([F, 1 + NPAIR], fp16, tag=f"pref{s % 2}")
                        nc.vector.memset(pref[:, 0:1], 0.0)
                        nc.vector._custom_dve(OP_SCAN, out=pref[:, 1:1 + NPAIR],
                                              in0=St[:], in1=Tt[:], s0=kt[:, 0:1])
                        sl = slice(s * 64, (s + 1) * 64)
                        ends = pref[:, 1:1 + NPAIR].rearrange(
                            "p (i j) -> p i j", j=N_AGENTS)[:, :, N_AGENTS - 1:N_AGENTS]
                        prevs = pref[:, 0:NPAIR].rearrange(
                            "p (i j) -> p i j", j=N_AGENTS)[:, :, 0:1]
                        nc.vector._custom_dve(
                            OP_DIFF,
                            out=agg[:, sl].rearrange("p (i o) -> p i o", o=1),
                            in0=ends, in1=prevs, s0=kt[:, 1:2])
                        # subtract self-edge (diagonal) messages
                        dtmp = small.tile([F, N_AGENTS], f32, tag=f"dt{s % 2}")
                        nc.vector.tensor_tensor(dtmp[:], St[:][:, 0:NPAIR:65],
                                                Tt[:][:, 0:NPAIR:65], op=OP.mult)
                        nc.vector.tensor_tensor(agg[:, sl], agg[:, sl], dtmp[:],
                                                op=OP.subtract)

                gaths = {}

                def block_stats(b):
                    # partial BN stats over this block's 4 samples; the b=0
                    # collective doubles as the firmware warmup and hides its
                    # latency under the b=1 half of the layer
                    sl = slice(b * NODES_PC // 2, (b + 1) * NODES_PC // 2)
                    s2 = small.tile([F, 2], f32, tag=f"s2_{b}")
                    nc.vector.tensor_reduce(s2[:, 0:1], agg[:, sl], axis=AX.X,
                                            op=OP.add)
                    trash = node.tile([F, NODES_PC // 2], f32, tag=f"trash{b}")
                    act(trash[:], agg[:, sl], AF.Square, accum_out=s2[:, 1:2])
                    dsum = nc.sync.dma_start(cc_in[(l, b)].ap()[:, :], s2[:])
                    ar = nc.gpsimd.collective_compute(
                        "AllGather", mybir.AluOpType.bypass,
                        replica_groups=[list(range(N_CORES))],
                        ins=[cc_in[(l, b)].ap().opt()],
                        outs=[cc_out[(l, b)].ap().opt()])
                    add_dep_helper(ar.ins, dsum.ins, reason="cc reads cc_in")
                    if l == 1 and b == 0:
                        add_dep_helper(ar.ins, warm_ar.ins,
                                       reason="warmup before first bn cc")
                    gath = small.tile([F, 2, N_CORES], f32, tag=f"gath{b}")
                    din = nc.sync.dma_start(
                        gath[:],
                        cc_out[(l, b)].ap().rearrange("(r p) c -> p c r", r=N_CORES))
                    add_dep_helper(din.ins, ar.ins, reason="dma reads cc_out")
                    gaths[b] = gath

                def ln_scan(samples):
                    for s in samples:
                        ln_block([s])
                        scan_block([s])

                mk_exp()
                sigma_block(0)
                mults_block(0)
                fill_half(BLK, 0, 0)
                fill_half(BLK, 0, 1)
                mults_block(1)
                ln_scan([0, 1, 2, 3])
                ln_block([4])
                ln_block([5])
                ln_block([6])
                block_stats(0)
                sigma_block(1, prefilled={(BLK, 0), (BLK, 1)})
                scan_block([4])
                scan_block([5])
                scan_block([6])

                # tail sample 7: halves pipelined (ln h1 overlaps scan h0)
                # to shorten the serial chain into the last BN collective
                St7, Ut7 = Ss[7], Us[7]
                Tt7 = pair.tile([F, NPAIR], bf16, tag=f"T{7 % 3}")
                Ts[7] = Tt7
                pref7 = pair.tile([F, 2 + NPAIR], fp16, tag="pref7")
                nc.vector.memset(pref7[:, 0:1], 0.0)
                nc.vector.memset(pref7[:, 2049:2050], 0.0)
                HN = NPAIR // 2
                for h in (0, 1):
                    hsl = slice(h * HN, (h + 1) * HN)
                    act(Tt7[:, hsl], Ut7[:, hsl], AF.Ln, bias=1.0)
                    base = h * (HN + 1)
                    nc.vector._custom_dve(
                        OP_SCAN, out=pref7[:, base + 1:base + 1 + HN],
                        in0=St7[:, hsl], in1=Tt7[:, hsl], s0=kt[:, 0:1])
                    ends = pref7[:, base + 1:base + 1 + HN].rearrange(
                        "p (i j) -> p i j", j=N_AGENTS)[:, :, N_AGENTS - 1:N_AGENTS]
                    prevs = pref7[:, base:base + HN].rearrange(
                        "p (i j) -> p i j", j=N_AGENTS)[:, :, 0:1]
                    asl = slice(7 * 64 + h * 32, 7 * 64 + (h + 1) * 32)
                    nc.vector._custom_dve(
                        OP_DIFF, out=agg[:, asl].rearrange("p (i o) -> p i o", o=1),
                        in0=ends, in1=prevs, s0=kt[:, 1:2])
                dtmp7 = small.tile([F, N_AGENTS], f32, tag="dt7")
                nc.vector.tensor_tensor(dtmp7[:], St7[:][:, 0:NPAIR:65],
                                        Tt7[:][:, 0:NPAIR:65], op=OP.mult)
                sl7 = slice(7 * 64, 8 * 64)
                nc.vector.tensor_tensor(agg[:, sl7], agg[:, sl7], dtmp7[:],
                                        op=OP.subtract)
                block_stats(1)

                gst = small.tile([F, 2], f32, tag="gst")
                nc.vector.tensor_reduce(gst[:], gaths[0][:], axis=AX.X, op=OP.add)
                gs1 = small.tile([F, 2], f32, tag="gs1")
                nc.vector.tensor_reduce(gs1[:], gaths[1][:], axis=AX.X, op=OP.add)
                nc.vector.tensor_tensor(gst[:], gst[:], gs1[:], op=OP.add)

                # ---- BN apply + residual + relu ----
                me2 = small.tile([F, 2], f32, tag="me2")
                nc.vector.tensor_scalar(me2[:], gst[:], 1.0 / N, None, op0=OP.mult)
                mean, ex2 = me2[:, 0:1], me2[:, 1:2]
                var = small.tile([F, 1], f32, tag="var")
                nc.vector.tensor_tensor(var[:], mean, mean, op=OP.mult)
                nc.vector.tensor_tensor(var[:], ex2, var[:], op=OP.subtract)
                lnv = small.tile([F, 1], f32, tag="lnv")
                act(lnv[:], var[:], AF.Ln, bias=eps_t[:])
                rstd = small.tile([F, 1], f32, tag="rstd")
                act(rstd[:], lnv[:], AF.Exp, bias=0.0, scale=-0.5)
                scal = small.tile([F, 1], f32, tag="scal")
                nc.vector.tensor_tensor(scal[:], rstd[:], wt[f"g{l}"], op=OP.mult)
                shneg = small.tile([F, 1], f32, tag="shneg")
                nc.vector.tensor_scalar(shneg[:], mean, scal[:, 0:1],
                                        wt[f"be{l}"],
                                        op0=OP.mult, op1=OP.subtract)
                nc.vector._custom_dve(OP_BNRES, out=x_out[:], in0=agg[:],
                                      in1=x_in[:], s0=scal[:, 0:1], s1=shneg[:, 0:1])

            x1 = io.tile([F, NODES_PC], f32, tag="x1")
            layer(1, xt, x1, xt)
            x1b = io.tile([F, NODES_PC], bf16, tag="x1b")
            nc.vector.tensor_scalar(x1b[:], x1[:], 1.0, None, op0=OP.mult)
            x2 = io.tile([F, NODES_PC], f32, tag="x2")
            layer(2, x1, x2, x1b)
            nc.sync.dma_start(yT.ap()[:, :], x2[:])

    nc.compile()
    return nc


def _get_nc():
    if "nc" not in _CACHE:
        _CACHE["nc"] = _build_nc()
    return _CACHE["nc"]


def _build_sel():
    """Sel[k, i*64+j] = (k<64 and k==i) or (k>=64 and k-64==j), bf16."""
    import ml_dtypes
    S = np.zeros((F, NPAIR), np.float32)
    for i in range(N_AGENTS):
        S[i, i * N_AGENTS:(i + 1) * N_AGENTS] = 1.0
    for j in range(N_AGENTS):
        S[64 + j, j::N_AGENTS] = 1.0
    return S.astype(ml_dtypes.bfloat16)


def _canonical_edge_ok(src, dst):
    idx = np.arange(N_AGENTS)
    rows = np.repeat(idx, N_AGENTS)
    cols = np.tile(idx, N_AGENTS)
    m = rows != cols
    rows, cols = rows[m], cols[m]
    offs = (np.arange(N_SAMPLES) * N_AGENTS)[:, None]
    csrc = (rows[None, :] + offs).ravel().astype(np.int64)
    cdst = (cols[None, :] + offs).ravel().astype(np.int64)
    if src.shape != csrc.shape:
        return False
    key = np.sort(src.astype(np.int64) * N + dst.astype(np.int64))
    ckey = np.sort(csrc * N + cdst)
    return bool(np.array_equal(key, ckey))


def _numpy_fallback(gnn_in, centers, src, dst, Ws_all):
    def sig(x):
        return 1.0 / (1.0 + np.exp(-x))

    def sp(x):
        return np.log1p(np.exp(-np.abs(x))) + np.maximum(x, 0.0)

    x = gnn_in.astype(np.float64)
    e = (centers[dst] - centers[src]).astype(np.float64)
    for (Wf, bf, Wsm, bs, g, be) in Ws_all:
        z = np.concatenate([x[dst], x[src], e], axis=-1)
        msg = sig(z @ Wf.T + bf) * sp(z @ Wsm.T + bs)
        agg = np.zeros_like(x)
        np.add.at(agg, dst, msg)
        mean = agg.mean(0)
        var = agg.var(0)
        agg = (agg - mean) / np.sqrt(var + BN_EPS) * g + be
        x = np.maximum(agg + x, 0.0)
    return x.astype(np.float32)


def _host_weights(Wf, bf, Ws, bs):
    """lhsT forms for the projection matmuls.

    WcAll layout [3, 4F]: cols 0:F   = c-part+bias of alpha (Wc3a)
                          cols F:2F  = c-part of beta (Wc3b)
                          cols 2F:3F = c-part+bias of gamma (Vc3g)
                          cols 3F:4F = c-part of delta (Vc3d)
    """
    WaT = np.ascontiguousarray(Wf[:, :F].T)
    WbT = np.ascontiguousarray(Wf[:, F:2 * F].T)
    Wc = Wf[:, 2 * F:2 * F + EDIM].T           # [2, 128]
    z = np.zeros((1, F), np.float32)
    Wc3a = np.concatenate([Wc, bf[None, :]], 0)
    Wc3b = np.concatenate([-Wc, z], 0)
    VaT = np.ascontiguousarray(Ws[:, :F].T)
    VbT = np.ascontiguousarray(Ws[:, F:2 * F].T)
    Vc = Ws[:, 2 * F:2 * F + EDIM].T
    Vc3g = np.concatenate([Vc, bs[None, :]], 0)
    Vc3d = np.concatenate([-Vc, z], 0)
    return WaT, WbT, VaT, VbT, (Wc3a, Wc3b, Vc3g, Vc3d)


def kernel(gnn_in, centers, src, dst,
           Wf1, bf1, Ws1, bs1, g1, be1,
           Wf2, bf2, Ws2, bs2, g2, be2,
           _trace=False, _tmpdir=None):
    gnn_in = np.ascontiguousarray(np.asarray(gnn_in, np.float32))
    centers = np.ascontiguousarray(np.asarray(centers, np.float32))
    src = np.asarray(src, np.int32)
    dst = np.asarray(dst, np.int32)
    args = [np.asarray(a, np.float32) for a in
            (Wf1, bf1, Ws1, bs1, g1, be1, Wf2, bf2, Ws2, bs2, g2, be2)]
    (Wf1, bf1, Ws1, bs1, g1, be1, Wf2, bf2, Ws2, bs2, g2, be2) = args

    if not _canonical_edge_ok(src, dst):
        import sys
        print("kernel.py: edge index is not block-fully-connected; numpy fallback",
              file=sys.stderr)
        return _numpy_fallback(gnn_in, centers, src, dst,
                               [(Wf1, bf1, Ws1, bs1, g1, be1),
                                (Wf2, bf2, Ws2, bs2, g2, be2)])

    import ml_dtypes
    from concourse import bass_utils

    nc = _get_nc()

    w1 = _host_weights(Wf1, bf1, Ws1, bs1)
    w2 = _host_weights(Wf2, bf2, Ws2, bs2)
    wmap = {}
    wc3 = {}
    raw = {}
    for l, w in ((1, w1), (2, w2)):
        for n, a in zip(("WaT", "WbT", "VaT", "VbT"), w[:4]):
            raw[f"{n}{l}"] = a.astype(np.float32)
        wc3[l] = w[4]
    wmap["gbe"] = np.ascontiguousarray(
        np.stack([g1, be1, g2, be2], 1).astype(np.float32))
    wmap["Sel"] = _build_sel()

    in_maps = []
    for k in range(N_CORES):
        sl = slice(k * NODES_PC, (k + 1) * NODES_PC)
        m = dict(wmap)
        m["xT"] = np.ascontiguousarray(gnn_in[sl].T).astype(ml_dtypes.bfloat16)
        c3k = np.concatenate([centers[sl].T, np.ones((1, NODES_PC), np.float32)], 0)
        for l in (1, 2):
            Wc3a, Wc3b, Vc3g, Vc3d = wc3[l]
            blocks = []
            for s in range(S_PC):
                cs = c3k[:, s * 64:(s + 1) * 64]             # [3, 64]
                blocks.append(np.concatenate([cs.T @ Wc3a, cs.T @ Wc3b], 0))
            cpab = np.concatenate(blocks, 1)
            cpgd = np.concatenate([Vc3g.T @ c3k, Vc3d.T @ c3k], 1)
            m[f"GD{l}"] = np.ascontiguousarray(np.concatenate(
                [raw[f"VaT{l}"], raw[f"VbT{l}"], cpgd], 1).astype(
                ml_dtypes.bfloat16))
            m[f"AB{l}"] = np.ascontiguousarray(np.concatenate(
                [raw[f"WaT{l}"], raw[f"WbT{l}"], cpab], 1).astype(
                ml_dtypes.bfloat16))
        in_maps.append(m)

    kw = {}
    if _trace:
        kw = dict(trace=True, tmpdir=_tmpdir)
    res = bass_utils.run_bass_kernel_spmd(nc, in_maps, core_ids=list(range(N_CORES)), **kw)

    out = np.empty((N, F), np.float32)
    for k in range(N_CORES):
        out[k * NODES_PC:(k + 1) * NODES_PC] = res.results[k]["yT"].T
    if _trace:
        _CACHE["last_res"] = res
    return out
